# revision 1
# baseline (speedup 1.0000x reference)
"""Trainium2 Bass kernel for nn_AverageAttn_62981400428866.

Reference computation (B=4, S=2048, D=1024):
    avg = cumavg_s(iV)                      # AAN lower-tri 1/(i+1) attention
    h   = relu(avg @ W1 + b1)
    ffn = h @ W2 + b2
    g   = sigmoid(concat([iQ, ffn], -1) @ Wg + bg)
    out = g[..., :D] * iQ + g[..., D:] * ffn

Sharding: 8 cores <- (batch b = c//2, seq half h = c%2); each core owns 1024
tokens. h=1 cores also receive the first-half iV block and reduce it on-device
to seed the cumulative-sum carry (h=0 cores get zeros there; same SPMD program).

Device layout: feature-major activations (features on partitions, tokens on the
free dim) so the mm1 -> mm2 -> gate chain needs no transposes: every matmul is
lhsT=weight-chunk [K=128, M=128], rhs=activation [K=128, N=512]. The cumulative
sum runs as block-triangular float32r matmuls against shifted-triangular
slices of a single "ramp" 0/1 constant; sub-diagonal all-ones blocks are
eliminated: the n=0 half's last scaled column, rescaled by 1/inv (carry2),
becomes the n=1 half's per-feature eviction bias. The h=1 half-prefix is
tree-reduced on DVE (free dim) plus one N=1 fp32 matmul per feature chunk
(cross-partition), and enters as the n=0 eviction bias. No serial carry
chain anywhere. iQ arrives host-transposed [D, tok] (pure layout prep,
like the weight rearranges and the host-side output transpose).
Heavy matmuls use float32r (full PE rate at N>=256, ~1.5e-4 rel err, fp32
storage, HW-validated). The final result is written feature-major [D, tok]
per core and transposed on the host during unshard.

The stages/attn_style/iqt_style/mm_bufs/reps parameters exist for perf
experiments only; grading uses the defaults via kernel().
"""

import numpy as np

B, S, D = 4, 2048, 1024
P = 128
NCORES = 8
TOK = S // 2          # tokens per core
TT = TOK // P         # token tiles per core
KC = D // P           # feature chunks
GC = 2 * D // P       # gate-dim chunks
NT = 512              # matmul moving free dim
NN = TOK // NT
RW = 3 * NT           # ramp constant width

_CACHE = {}


def _build_nc(reps=1, stages=("prefix", "iqt", "attn", "mm1", "mm2", "gate", "final"),
              attn_style="block", iqt_style="single", mm_bufs=7, attn_trim=True):
    from concourse import bacc
    import concourse.mybir as mybir
    from concourse.tile import TileContext

    f32 = mybir.dt.float32
    f32r = mybir.dt.float32r
    AF = mybir.ActivationFunctionType
    ALU = mybir.AluOpType

    nc = bacc.Bacc(None, target_bir_lowering=False)
    iq_d = nc.dram_tensor("iq", [D, TOK], f32r, kind="ExternalInput")
    iv_d = nc.dram_tensor("iv", [TOK, D], f32r, kind="ExternalInput")
    ivp_d = nc.dram_tensor("ivp", [TOK, D], f32, kind="ExternalInput")
    w1_d = nc.dram_tensor("w1", [KC, P, KC, P], f32r, kind="ExternalInput")
    w2_d = nc.dram_tensor("w2", [KC, P, KC, P], f32r, kind="ExternalInput")
    wg_d = nc.dram_tensor("wg", [GC, P, GC, P], f32r, kind="ExternalInput")
    b1_d = nc.dram_tensor("b1c", [P, KC], f32, kind="ExternalInput")
    b2_d = nc.dram_tensor("b2c", [P, KC], f32, kind="ExternalInput")
    bg_d = nc.dram_tensor("bgc", [P, GC], f32, kind="ExternalInput")
    inv_d = nc.dram_tensor("invrep", [P, TOK], f32, kind="ExternalInput")
    ramp_d = nc.dram_tensor("ramp", [P, RW], f32r, kind="ExternalInput")
    out_d = nc.dram_tensor("outT", [D, TOK], f32, kind="ExternalOutput")
    scratch_d = (
        nc.dram_tensor("scratch", [D, TOK], f32, kind="Internal")
        if reps > 1
        else None
    )

    with TileContext(nc) as tc:
        with (
            tc.tile_pool(name="big", bufs=4) as big,
            tc.tile_pool(name="wpool", bufs=3) as wpool,
            tc.tile_pool(name="tok", bufs=4) as tokp,
            tc.tile_pool(name="gp", bufs=2) as gpool,
            tc.tile_pool(name="smp", bufs=1) as smallp,
            tc.tile_pool(name="const", bufs=1) as constp,
            tc.tile_pool(name="pre_ps", bufs=1, space="PSUM") as pre_ps,
            tc.tile_pool(name="mm_ps", bufs=mm_bufs, space="PSUM") as mm_ps,
        ):
          for rep in range(reps):
            out_rep = out_d if rep == 0 else scratch_d
            ramp_t = constp.tile([P, RW], f32r)
            nc.sync.dma_start(ramp_t[:], ramp_d[:])
            inv_t = constp.tile([P, TOK], f32)
            nc.sync.dma_start(inv_t[:], inv_d[:])
            b1_t = constp.tile([P, KC], f32)
            nc.sync.dma_start(b1_t[:], b1_d[:])
            b2_t = constp.tile([P, KC], f32)
            nc.sync.dma_start(b2_t[:], b2_d[:])
            bg_t = constp.tile([P, GC], f32)
            nc.sync.dma_start(bg_t[:], bg_d[:])

            iv_r = iv_d.rearrange("(t p) d -> p t d", p=P)
            ivp_r = ivp_d.rearrange("(t p) d -> p t d", p=P)

            V = big.tile([P, TT, D], f32r, tag="big")
            Vp = big.tile([P, TT, D], f32, tag="big")
            # Vp first: it heads the critical path (DVE tree -> carry gates
            # every attn eviction). V loads per feature-column stripe so attn
            # dc=0 starts after the first 512KB instead of the full 4MB.
            for t in range(TT):
                nc.sync.dma_start(Vp[:, t], ivp_r[:, t])
            for dc in range(KC):
                sl = slice(dc * P, (dc + 1) * P)
                nc.sync.dma_start(V[:, :, sl], iv_r[:, :, sl])

            # half-prefix: carry[p, dc] = sum_tok iVpre[tok, dc*P+p].
            # Token-tile partial sums tree-reduce on DVE (free dim), then a
            # single N=1 fp32 matmul per feature chunk crosses partitions.
            carry = smallp.tile([P, KC], f32)
            if "prefix" not in stages and "attn" in stages:
                nc.any.memzero(carry[:])
            pre = pre_ps.tile([P, KC], f32)
            ones_col = ramp_t[:, RW - 1 : RW].bitcast(f32)
            if "prefix" in stages:
                pa_ = tokp.tile([P, D], f32, tag="pfx", name="pa_", bufs=2)
                pb_ = tokp.tile([P, D], f32, tag="pfx", name="pb_", bufs=2)
                nc.vector.tensor_tensor(pa_[:], Vp[:, 0], Vp[:, 1], ALU.add)
                nc.vector.tensor_tensor(pb_[:], Vp[:, 2], Vp[:, 3], ALU.add)
                nc.vector.tensor_tensor(pa_[:], pa_[:], pb_[:], ALU.add)
                nc.vector.tensor_tensor(pb_[:], Vp[:, 4], Vp[:, 5], ALU.add)
                nc.vector.tensor_tensor(pa_[:], pa_[:], pb_[:], ALU.add)
                nc.vector.tensor_tensor(pb_[:], Vp[:, 6], Vp[:, 7], ALU.add)
                nc.vector.tensor_tensor(pa_[:], pa_[:], pb_[:], ALU.add)
                for dc in range(KC):
                    nc.tensor.matmul(
                        pre[:, dc : dc + 1],
                        pa_[:, dc * P : (dc + 1) * P],
                        ones_col,
                        start=True,
                        stop=True,
                    )
                nc.any.tensor_copy(carry[:], pre[:])

            # cumulative average, feature-major, via block-triangular matmuls:
            # avgT[p, dc, n] = (carry[p,dc] + sum_{k<=n} V[k, dc*P+p]) * inv[n]
            # rhs blocks are slices of the ramp constant (ones / shifted tri).
            avgT = big.tile([P, KC, TOK], f32r, tag="big")
            if "attn" not in stages and "mm1" in stages:
                nc.any.memzero(avgT[:])
            for dc in range(KC if ("attn" in stages and attn_style == "chain") else 0):
                for t in range(TT):
                    pa = mm_ps.tile([P, NT], f32, tag="mps", name="pa")
                    nc.tensor.matmul(
                        pa[:, :P],
                        V[:, t, dc * P : (dc + 1) * P].bitcast(f32),
                        ramp_t[:, NT : NT + P].bitcast(f32),
                        start=True,
                        stop=True,
                    )
                    sl = slice(t * P, (t + 1) * P)
                    nc.scalar.activation(
                        avgT[:, dc, sl], pa[:, :P], AF.Identity,
                        bias=carry[:, dc : dc + 1],
                    )
                    nc.vector.tensor_tensor(
                        avgT[:, dc, sl], avgT[:, dc, sl], inv_t[:, sl], ALU.mult
                    )
                    if t < TT - 1:
                        nc.vector.tensor_tensor(
                            carry[:, dc : dc + 1],
                            carry[:, dc : dc + 1],
                            pa[:, P - 1 : P],
                            ALU.add,
                        )
            # recip511[p] = pos(511)+1 = 1/inv[511]; used to rebuild the raw
            # first-half cumsum from the scaled avgT column 511 so the n=1
            # blocks skip their all-ones matmuls entirely.
            recip511 = smallp.tile([P, 1], f32, name="recip511")
            carry2 = smallp.tile([P, KC], f32, name="carry2")
            if "attn" in stages and attn_style == "block":
                nc.vector.reciprocal(recip511[:], inv_t[:, NT - 1 : NT])
            for dc in range(KC if ("attn" in stages and attn_style == "block") else 0):
                pss = [
                    mm_ps.tile([P, NT], f32, tag="mps", name=f"aps{i}")
                    for i in range(NN)
                ]
                for kt in range(TT):
                    for n in range(NN):
                        lo = n * (TT // NN)  # first diagonal tile of this n-block
                        if kt < lo or kt >= lo + TT // NN:
                            continue  # off-diagonal (ones handled via carry2)
                        j = kt - lo
                        if attn_trim:
                            # block j is zero in columns < j*P: stream only the
                            # live range, writing at a psum column offset
                            rhs = ramp_t[:, NT : 2 * NT - j * P]
                            outp = pss[n][:, j * P : NT]
                        else:
                            rhs = ramp_t[:, NT - j * P : 2 * NT - j * P]
                            outp = pss[n][:]
                        nc.tensor.matmul(
                            outp,
                            V[:, kt, dc * P : (dc + 1) * P],
                            rhs,
                            start=(kt == lo),
                            stop=(kt == lo + TT // NN - 1),
                        )
                nc.scalar.activation(
                    avgT[:, dc, 0:NT], pss[0][:], AF.Identity,
                    bias=carry[:, dc : dc + 1],
                )
                nc.vector.tensor_tensor(
                    avgT[:, dc, 0:NT], avgT[:, dc, 0:NT], inv_t[:, 0:NT], ALU.mult
                )
                nc.scalar.activation(
                    carry2[:, dc : dc + 1], avgT[:, dc, NT - 1 : NT], AF.Copy,
                    scale=recip511[:],
                )
                nc.scalar.activation(
                    avgT[:, dc, NT : 2 * NT], pss[1][:], AF.Identity,
                    bias=carry2[:, dc : dc + 1],
                )
                nc.vector.tensor_tensor(
                    avgT[:, dc, NT : 2 * NT], avgT[:, dc, NT : 2 * NT],
                    inv_t[:, NT : 2 * NT], ALU.mult,
                )

            # mm1: hT = relu(W1^T @ avgT + b1)
            hT = big.tile([P, KC, TOK], f32r, tag="big")
            if "mm1" not in stages and "mm2" in stages:
                nc.any.memzero(hT[:])
            for mc in range(KC if "mm1" in stages else 0):
                w = wpool.tile([P, GC, P], f32r, tag="w")
                nc.sync.dma_start(w[:, :KC], w1_d[mc])
                pss = [mm_ps.tile([P, NT], f32, tag="mps", name=f"mps{i}") for i in range(NN)]
                for kc in range(KC):
                    for n in range(NN):
                        nsl = slice(n * NT, (n + 1) * NT)
                        nc.tensor.matmul(
                            pss[n][:],
                            w[:, kc],
                            avgT[:, kc, nsl],
                            start=(kc == 0),
                            stop=(kc == KC - 1),
                        )
                for n in range(NN):
                    nsl = slice(n * NT, (n + 1) * NT)
                    nc.scalar.activation(
                        hT[:, mc, nsl], pss[n][:], AF.Relu, bias=b1_t[:, mc : mc + 1]
                    )

            # iQ arrives host-transposed [D, TOK] (layout prep, like the
            # weight rearranges); stripe DMAs straight into feature-major.
            iQT = big.tile([P, KC, TOK], f32r, tag="big")
            if "iqt" in stages:
                iq_r = iq_d.rearrange("(dc p) n -> p dc n", p=P)
                for dc in range(KC):
                    nc.sync.dma_start(iQT[:, dc], iq_r[:, dc])
            elif "gate" in stages or "final" in stages:
                nc.any.memzero(iQT[:])

            # mm2: ffnT = W2^T @ hT + b2
            ffnT = big.tile([P, KC, TOK], f32r, tag="big")
            if "mm2" not in stages and ("gate" in stages or "final" in stages):
                nc.any.memzero(ffnT[:])
            for mc in range(KC if "mm2" in stages else 0):
                w = wpool.tile([P, GC, P], f32r, tag="w")
                nc.sync.dma_start(w[:, :KC], w2_d[mc])
                pss = [mm_ps.tile([P, NT], f32, tag="mps", name=f"mps{i}") for i in range(NN)]
                for kc in range(KC):
                    for n in range(NN):
                        nsl = slice(n * NT, (n + 1) * NT)
                        nc.tensor.matmul(
                            pss[n][:],
                            w[:, kc],
                            hT[:, kc, nsl],
                            start=(kc == 0),
                            stop=(kc == KC - 1),
                        )
                for n in range(NN):
                    nsl = slice(n * NT, (n + 1) * NT)
                    nc.scalar.activation(
                        ffnT[:, mc, nsl], pss[n][:], AF.Identity, bias=b2_t[:, mc : mc + 1]
                    )

            # gate + output: per D-chunk dc, gates gc=dc (igate) and dc+KC (fgate)
            for dc in range(KC if ("gate" in stages or "final" in stages) else 0):
                g2 = gpool.tile([P, 2, TOK], f32, tag="g")
                if "gate" not in stages and "final" in stages:
                    nc.any.memzero(g2[:])
                for gi, gc in enumerate((dc, dc + KC) if "gate" in stages else ()):
                    wg = wpool.tile([P, GC, P], f32r, tag="w")
                    nc.sync.dma_start(wg[:], wg_d[gc])
                    pss = [mm_ps.tile([P, NT], f32, tag="mps", name=f"mps{i}") for i in range(NN)]
                    for kc in range(GC):
                        for n in range(NN):
                            nsl = slice(n * NT, (n + 1) * NT)
                            rhs = (
                                iQT[:, kc, nsl] if kc < KC else ffnT[:, kc - KC, nsl]
                            )
                            nc.tensor.matmul(
                                pss[n][:],
                                wg[:, kc],
                                rhs,
                                start=(kc == 0),
                                stop=(kc == GC - 1),
                            )
                    for n in range(NN):
                        nsl = slice(n * NT, (n + 1) * NT)
                        nc.scalar.activation(
                            g2[:, gi, nsl], pss[n][:], AF.Sigmoid, bias=bg_t[:, gc : gc + 1]
                        )
                if "final" not in stages:
                    continue
                outc = tokp.tile([P, TOK], f32, tag="tok")
                tmp = tokp.tile([P, TOK], f32, tag="tok")
                for n in range(NN):
                    nsl = slice(n * NT, (n + 1) * NT)
                    nc.vector.tensor_tensor(
                        outc[:, nsl], g2[:, 0, nsl], iQT[:, dc, nsl], ALU.mult
                    )
                    nc.vector.tensor_tensor(
                        tmp[:, nsl], g2[:, 1, nsl], ffnT[:, dc, nsl], ALU.mult
                    )
                    nc.vector.tensor_tensor(
                        outc[:, nsl], outc[:, nsl], tmp[:, nsl], ALU.add
                    )
                    nc.sync.dma_start(
                        out_rep[dc * P : (dc + 1) * P, nsl], outc[:, nsl]
                    )

    nc.compile()
    return nc


def _get_nc(reps=1):
    key = ("nc", reps)
    if key not in _CACHE:
        _CACHE[key] = _build_nc(reps)
    return _CACHE[key]


def _host_inputs(iQ, iV, W1, b1, W2, b2, Wg, bg):
    iQ = np.asarray(iQ, np.float32)
    iV = np.asarray(iV, np.float32)
    w1_rr = np.ascontiguousarray(
        np.asarray(W1, np.float32).reshape(KC, P, KC, P).transpose(2, 1, 0, 3)
    )
    w2_rr = np.ascontiguousarray(
        np.asarray(W2, np.float32).reshape(KC, P, KC, P).transpose(2, 1, 0, 3)
    )
    wg_rr = np.ascontiguousarray(
        np.asarray(Wg, np.float32).reshape(GC, P, GC, P).transpose(2, 1, 0, 3)
    )
    b1c = np.ascontiguousarray(np.asarray(b1, np.float32).reshape(KC, P).T)
    b2c = np.ascontiguousarray(np.asarray(b2, np.float32).reshape(KC, P).T)
    bgc = np.ascontiguousarray(np.asarray(bg, np.float32).reshape(GC, P).T)
    ramp = (
        np.arange(P, dtype=np.int64)[:, None] <= np.arange(RW, dtype=np.int64)[None, :] - NT
    ).astype(np.float32)
    zeros_pre = np.zeros((TOK, D), np.float32)

    in_maps = []
    for c in range(NCORES):
        b, h = divmod(c, 2)
        sl = slice(h * TOK, (h + 1) * TOK)
        inv = (
            np.float32(1.0)
            / np.arange(h * TOK + 1, h * TOK + TOK + 1, dtype=np.float32)
        )
        in_maps.append(
            {
                "iq": np.ascontiguousarray(iQ[b, sl].T),
                "iv": np.ascontiguousarray(iV[b, sl]),
                "ivp": np.ascontiguousarray(iV[b, :TOK]) if h else zeros_pre,
                "w1": w1_rr,
                "w2": w2_rr,
                "wg": wg_rr,
                "b1c": b1c,
                "b2c": b2c,
                "bgc": bgc,
                "invrep": np.ascontiguousarray(np.broadcast_to(inv, (P, TOK))),
                "ramp": ramp,
            }
        )
    return in_maps


def _gather(results):
    out = np.empty((B, S, D), np.float32)
    for c in range(NCORES):
        b, h = divmod(c, 2)
        out[b, h * TOK : (h + 1) * TOK, :] = results[c]["outT"].T
    return out


def kernel(iQ, iV, W1, b1, W2, b2, Wg, bg):
    from concourse.bass_utils import run_bass_kernel_spmd

    nc = _get_nc()
    in_maps = _host_inputs(iQ, iV, W1, b1, W2, b2, Wg, bg)
    res = run_bass_kernel_spmd(nc, in_maps, core_ids=list(range(NCORES)))
    return _gather(res.results)

